# revision 1
# baseline (speedup 1.0000x reference)
"""Trainium2 Bass kernel for CGC3 MoE routing (nn_CGC3_48163763257633).

Full-input contract: kernel(**inputs) takes the unsharded inputs and returns
the full [B, T+1, O] output. Internally: data-parallel over batch across 8
NeuronCores (weights replicated), no collectives.

Per-core program (B_local=1024):
  - all matmuls in float32r (TF32-like precision, full PE rate at N>=256;
    measured ~240 ns per [128x128]@[128x512] MM, ~2e-4 whole-kernel rel err)
  - layer 1 computed transposed: hT[e] = relu(W1[e].T-slices @ xT) -> [H, B] layout
    so layer 2 (contraction over H) needs no on-chip transposes
  - layer 2: eo[e] = relu(hT[e].T @ W2[e]) -> [B, O] layout, drained from PSUM by
    ACT while DVE applies the 2-3 gate scalars per expert directly into the
    3 output-group accumulators (scalar_tensor_tensor FMA)
  - gates: one [128,20] matmul per batch tile (Wg[0]|Wg[1]|Wgs concatenated),
    softmax per column-group on the free dim
"""

import sys

sys.path.insert(0, "/opt/trn_rl_repo")

import numpy as np

import concourse.bass as bass  # noqa: F401  (AP helpers)
import concourse.mybir as mybir
import concourse.tile as tile
from concourse import bacc
from concourse.bass_utils import run_bass_kernel_spmd

# ---- problem constants (hardcoded per contract) ----
B, D = 8192, 1024
H, O = 1024, 512
T, E_T, E_S = 2, 2, 4
E = T * E_T + E_S          # 8 experts
NG = T + 1                 # 3 output groups (task0, task1, shared)
GC = T * (E_T + E_S) + E   # 20 gate columns: 6 + 6 + 8
NCORES = 8
BL = B // NCORES           # 1024 rows per core
P = 128
KT = D // P                # 8 k-tiles over D
HT = H // P                # 8 partition-tiles over H
BT = BL // P               # 8 batch tiles per core
BH = BL // 512             # 2 batch halves (layer-1 free dim)

F32 = mybir.dt.float32
F32R = mybir.dt.float32r
RELU = mybir.ActivationFunctionType.Relu
EXP = mybir.ActivationFunctionType.Exp
MAX = mybir.AluOpType.max
MULT = mybir.AluOpType.mult
ADD = mybir.AluOpType.add
AXX = mybir.AxisListType.X


def _expert_contribs():
    """(group, gate-column) pairs each expert feeds.

    Task i's softmax columns order its own E_T experts first, then the E_S
    shared experts. The shared tower (group NG-1) covers all E experts.
    """
    contribs = {e: [] for e in range(E)}
    for i in range(T):
        base = i * (E_T + E_S)
        for j, e in enumerate(
            list(range(i * E_T, (i + 1) * E_T)) + list(range(E - E_S, E))
        ):
            contribs[e].append((i, base + j))
    shared_base = T * (E_T + E_S)
    for e in range(E):
        contribs[e].append((T, shared_base + e))
    return contribs


def _build_program(has_b2: bool, repeat: int = 1, mm_dt: str = "f32r"):
    """repeat>1 replicates the whole compute body inside one NEFF (timing
    harness only — output is then wrong; slope over repeat isolates per-run
    exec time from the large fixed axon dispatch overhead)."""
    BF16 = mybir.dt.bfloat16
    if mm_dt == "f32r":
        WDT = ADT = F32R          # weights / activations (xT, hT)
    elif mm_dt == "bf16":
        WDT = ADT = BF16
    elif mm_dt == "mixed":
        WDT, ADT = BF16, F32R     # bf16 stationary (pipelined LDW), f32r moving
    else:
        raise ValueError(mm_dt)
    nc = bacc.Bacc("TRN2", target_bir_lowering=False, debug=False,
                   num_devices=NCORES)

    xT_d = nc.dram_tensor("xT", [D, BL], ADT, kind="ExternalInput").ap()
    w1_d = nc.dram_tensor("W1", [E, D, H], WDT, kind="ExternalInput").ap()
    w2_d = nc.dram_tensor("W2", [E, H, O], WDT, kind="ExternalInput").ap()
    wcat_d = nc.dram_tensor("Wcat", [D, GC], WDT, kind="ExternalInput").ap()
    b1_d = nc.dram_tensor("b1r", [P, E * HT], F32, kind="ExternalInput").ap()
    if has_b2:
        b2_d = nc.dram_tensor("b2", [E, O], F32, kind="ExternalInput").ap()
    out_d = nc.dram_tensor("out", [BL, NG, O], F32, kind="ExternalOutput").ap()

    contribs = _expert_contribs()

    with tile.TileContext(nc) as tc:
        with (
            tc.tile_pool(name="xt", bufs=KT) as xt_pool,
            tc.tile_pool(name="w1", bufs=12) as w1_pool,
            tc.tile_pool(name="w2", bufs=12) as w2_pool,
            tc.tile_pool(name="ht", bufs=10 if not has_b2 else 9) as ht_pool,
            tc.tile_pool(name="eo", bufs=4) as eo_pool,
            tc.tile_pool(name="acc", bufs=NG * BT) as acc_pool,
            tc.tile_pool(name="gate", bufs=BT) as gate_pool,
            tc.tile_pool(name="small", bufs=1) as small_pool,
            tc.tile_pool(name="b2p", bufs=2) as b2_pool,
            tc.tile_pool(name="tmp", bufs=6) as tmp_pool,
            tc.tile_pool(name="psmm", bufs=6, space="PSUM") as psmm_pool,
            tc.tile_pool(name="psg", bufs=2, space="PSUM") as psg_pool,
        ):
            # ---- persistent loads ----
            xts = []
            for kt in range(KT):
                t = xt_pool.tile([P, BL], ADT)
                nc.sync.dma_start(out=t, in_=xT_d[kt * P:(kt + 1) * P, :])
                xts.append(t)

            b1t = small_pool.tile([P, E * HT], F32)
            nc.sync.dma_start(out=b1t, in_=b1_d)

            wcat_t = small_pool.tile([P, KT, GC], WDT)
            nc.sync.dma_start(
                out=wcat_t, in_=wcat_d.rearrange("(k p) g -> p k g", p=P)
            )


            for _rep in range(repeat):
                # ---- gates: G[b, 0:20] then per-group softmax on the free dim ----
                gts = []
                for bt in range(BT):
                    gps = psg_pool.tile([P, GC], F32)
                    for kt in range(KT):
                        nc.tensor.matmul(
                            gps,
                            xts[kt][:, bt * P:(bt + 1) * P],
                            wcat_t[:, kt, :],
                            start=(kt == 0),
                            stop=(kt == KT - 1),
                        )
                    gt = gate_pool.tile([P, GC], F32)
                    for gi in range(T + 1):
                        c0 = gi * (E_T + E_S) if gi < T else T * (E_T + E_S)
                        c1 = c0 + (E_T + E_S if gi < T else E)
                        nm = tmp_pool.tile([P, 1], F32)
                        nc.vector.tensor_reduce(
                            out=nm, in_=gps[:, c0:c1], axis=AXX, op=MAX, negate=True
                        )
                        es = tmp_pool.tile([P, 1], F32)
                        nc.scalar.activation(
                            out=gt[:, c0:c1], in_=gps[:, c0:c1], func=EXP,
                            bias=nm, scale=1.0, accum_out=es,
                        )
                        rs = tmp_pool.tile([P, 1], F32)
                        nc.vector.reciprocal(out=rs, in_=es)
                        nc.vector.tensor_scalar_mul(gt[:, c0:c1], gt[:, c0:c1], rs)
                    gts.append(gt)

                # ---- experts ----
                acc_tiles = {}
                for e in range(E):
                    b2bt = None
                    if has_b2:
                        # b2[e] broadcast across partitions (DMA stride-0)
                        b2bt = b2_pool.tile([P, O], F32, tag="b2")
                        b2_row = b2_d[e, :]
                        b2_bcast = bass.AP(
                            tensor=b2_row.tensor,
                            offset=b2_row.offset,
                            ap=[[0, P]] + [list(a) for a in b2_row.ap],
                        )
                        nc.sync.dma_start(out=b2bt, in_=b2_bcast)
                    w1ts = []
                    for kt in range(KT):
                        t = w1_pool.tile([P, H], WDT)
                        nc.sync.dma_start(out=t, in_=w1_d[e, kt * P:(kt + 1) * P, :])
                        w1ts.append(t)
                    w2ts = []
                    for ht in range(HT):
                        t = w2_pool.tile([P, O], WDT)
                        nc.sync.dma_start(out=t, in_=w2_d[e, ht * P:(ht + 1) * P, :])
                        w2ts.append(t)

                    # layer 1: hT[e] = relu(x @ W1[e] (+ b1))^T  -> [H, BL] layout
                    # kt-outer so each W1 stationary serves both batch halves
                    # (consecutive same-lhsT f32r matmuls skip the weight load)
                    hts = []
                    for ht in range(HT):
                        htile = ht_pool.tile([P, BL], ADT)
                        hpss = []
                        for bh in range(BH):
                            hps = psmm_pool.tile([P, 512], F32, tag="mm")
                            hpss.append(hps)
                        for kt in range(KT):
                            for bh in range(BH):
                                nc.tensor.matmul(
                                    hpss[bh],
                                    w1ts[kt][:, ht * P:(ht + 1) * P],
                                    xts[kt][:, bh * 512:(bh + 1) * 512],
                                    start=(kt == 0),
                                    stop=(kt == KT - 1),
                                )
                        for bh in range(BH):
                            nc.scalar.activation(
                                out=htile[:, bh * 512:(bh + 1) * 512], in_=hpss[bh],
                                func=RELU, bias=b1t[:, e * HT + ht:e * HT + ht + 1],
                                scale=1.0,
                            )
                        hts.append(htile)

                    # layer 2 + fused gating drain
                    for bt in range(BT):
                        ops = psmm_pool.tile([P, O], F32, tag="mm")
                        for ht in range(HT):
                            nc.tensor.matmul(
                                ops,
                                hts[ht][:, bt * P:(bt + 1) * P],
                                w2ts[ht],
                                start=(ht == 0),
                                stop=(ht == HT - 1),
                            )
                        eot = eo_pool.tile([P, O], F32, tag="eot")
                        if has_b2:
                            pre = eo_pool.tile([P, O], F32, tag="eot")
                            nc.vector.scalar_tensor_tensor(
                                out=pre, in0=ops, scalar=1.0, in1=b2bt,
                                op0=MULT, op1=ADD,
                            )
                            nc.scalar.activation(out=eot, in_=pre, func=RELU)
                        else:
                            nc.scalar.activation(out=eot, in_=ops, func=RELU)

                        for (grp, col) in contribs[e]:
                            key = (grp, bt)
                            sc = gts[bt][:, col:col + 1]
                            if key not in acc_tiles:
                                at = acc_pool.tile([P, O], F32)
                                acc_tiles[key] = at
                                nc.vector.tensor_scalar_mul(at, eot, sc)
                            else:
                                nc.vector.scalar_tensor_tensor(
                                    out=acc_tiles[key], in0=eot, scalar=sc,
                                    in1=acc_tiles[key], op0=MULT, op1=ADD,
                                )

                # ---- writeback ----
                for (grp, bt), at in acc_tiles.items():
                    nc.sync.dma_start(
                        out=out_d[bt * P:(bt + 1) * P, grp, :], in_=at
                    )

    nc.compile()
    return nc


_CACHE = {}


MM_DT = "f32r"   # kernel-wide matmul dtype: "f32r" or "bf16"


def _get_program(has_b2: bool):
    return _get_program_rep(has_b2, 1)


def _get_program_rep(has_b2: bool, repeat: int, mm_dt: str | None = None):
    key = (has_b2, repeat, mm_dt or MM_DT)
    if key not in _CACHE:
        _CACHE[key] = _build_program(has_b2, repeat, mm_dt or MM_DT)
    return _CACHE[key]


def make_in_maps(x, W1, b1, W2, b2, Wg, Wgs, mm_dt: str | None = None):
    """Host-side shard/layout prep -> per-core input dicts."""
    import ml_dtypes
    mdt = mm_dt or MM_DT
    w_cast = np.float32 if mdt == "f32r" else ml_dtypes.bfloat16
    x_cast = ml_dtypes.bfloat16 if mdt == "bf16" else np.float32
    x = np.ascontiguousarray(x, dtype=np.float32)
    W1 = np.ascontiguousarray(W1, dtype=np.float32)
    W2 = np.ascontiguousarray(W2, dtype=np.float32)
    b1 = np.asarray(b1, dtype=np.float32)
    b2 = np.asarray(b2, dtype=np.float32)
    Wcat = np.concatenate(
        [Wg[i] for i in range(T)] + [Wgs], axis=1
    ).astype(np.float32)  # [D, 20]
    b1r = np.ascontiguousarray(
        b1.reshape(E, HT, P).transpose(2, 0, 1).reshape(P, E * HT)
    )
    has_b2 = bool(np.any(b2))
    W1c = W1.astype(w_cast)
    W2c = W2.astype(w_cast)
    Wcatc = Wcat.astype(w_cast)
    in_maps = []
    for c in range(NCORES):
        xs = x[c * BL:(c + 1) * BL]
        m = {
            "xT": np.ascontiguousarray(xs.T).astype(x_cast),
            "W1": W1c,
            "W2": W2c,
            "Wcat": Wcatc,
            "b1r": b1r,
        }
        if has_b2:
            m["b2"] = b2
        in_maps.append(m)
    return in_maps, has_b2


def kernel(x, W1, b1, W2, b2, Wg, Wgs):
    in_maps, has_b2 = make_in_maps(x, W1, b1, W2, b2, Wg, Wgs)
    nc = _get_program(has_b2)
    res = run_bass_kernel_spmd(nc, in_maps, list(range(NCORES)))
    return np.concatenate([r["out"] for r in res.results], axis=0)

